# revision 15
# baseline (speedup 1.0000x reference)
"""Two-layer GAT + linear head + log_softmax on 8 Trainium2 NeuronCores.

Edge-major design:
  - Nodes sharded 12500/core by id range (natural order). Each core owns the
    edges whose aggregation row (src = edge[0]) is in its shard.
  - Feature tables: h_ext rows [feat | 1.0 | s_dst] in bf16, one row per node,
    in a 4-quarter global table (8 core regions of 16384 rows; a quarter =
    2 regions = 32768 rows so quarter-local indices fit dma_gather's int16).
    Computed shard-locally (dense matmul), AllGathered.
  - Edges are sorted by (src tile, dst quarter, src) and gathered DENSELY
    (no per-node slot padding) with one dma_gather per (tile, quarter):
    thousands of descriptors per SWDGE call instead of 128 per
    indirect_dma_start call — this removes the descriptor-generation
    bottleneck of the previous version.
  - Per 128-edge column, aggregation uses a [128, 128] 0/1 selection matrix
    (edge partition -> node) built on DVE via is_equal against an iota,
    scaled by the edge softmax weights; one PE matmul per column
    segment-sums weights*[feat|1] into a per-node psum, yielding both the
    weighted feature sum and z (softmax denominator, via the 1.0 column).
    Softmax is computed without max-subtraction (scores are O(10)).
  - s_src is broadcast node->edge with a PE matvec per column using the
    transposed selection matrix.
  - Column counts are unified across cores (the Bass program is shared);
    per-core data differs only in inputs.

Self-contained: hardcodes N=100000, E=3200000, 8 cores.
"""

import numpy as np
import ml_dtypes

NC_CORES = 8
P = 128
N = 100000
SH = N // NC_CORES           # 12500 nodes per core
T = (SH + P - 1) // P        # 98 tiles
POS = T * P                  # 12544 padded positions
REG = 16384                  # table rows per core region
QROWS = 32768                # rows per quarter (2 regions)
NQ = 4
PADLOC = 12500               # quarter-local index of a pad row
ALPHA = 0.2
NEG = -1.0e30
ELEM0, STEP0, RH0 = 258, 384, 257    # L0 row [256 feat | 1 | s_dst]
ELEM1, STEP1, RH1 = 130, 256, 129    # L1 row [128 feat | 1 | s_dst]

BF16 = ml_dtypes.bfloat16


def _dma_gather_raw(gp, out_ap, in_ap, idxs_ap, num_idxs, num_idxs_reg,
                    elem_size, elem_step, queue_num=0):
    """dma_gather minus the elem_size%256 assert (non-transpose path only).

    The %256 restriction in bass.BassGpSimd.dma_gather is a transpose-mode
    requirement; the non-transpose ucode uses byte-granular packet lengths.
    Row stride (elem_step) must still be a multiple of 256 bytes.
    gathered = in[idxs, :elem_size]; out[i%128, i//128, :] = row idxs[i].
    """
    import concourse.bass as bass
    import concourse.mybir as mybir
    from concourse.ap_utils import ap_is_contiguous

    gp._assert_queue_num(queue_num)
    assert idxs_ap.dtype == mybir.dt.int16
    assert in_ap.space == bass.MemorySpace.DRAM
    assert idxs_ap.space == bass.MemorySpace.SBUF
    assert out_ap.space == bass.MemorySpace.SBUF
    assert in_ap.dtype == out_ap.dtype
    assert ap_is_contiguous(out_ap.ap[1:])
    assert ap_is_contiguous(idxs_ap.ap[1:])
    assert in_ap.ap[-1][1] == elem_size
    assert out_ap.ap[-1][1] == elem_size
    assert in_ap.ap[0][0] == elem_step
    stride_bytes = elem_step * mybir.dt.size(in_ap.dtype)
    assert stride_bytes % 256 == 0
    stride_bytes_256 = stride_bytes // 256
    assert stride_bytes_256 < 256

    _in_ap = gp.lower_ap_dma(in_ap, for_custom_bir_dma=True)
    _idxs_ap = gp.lower_ap(idxs_ap)
    _out_ap = gp.lower_ap(out_ap)
    return gp.add_instruction(
        mybir.InstDMAGatherAnt(
            name=gp.bass.get_next_instruction_name(),
            ins=[*_in_ap, _idxs_ap, gp.lower_val_access(gp.to_reg(num_idxs_reg))],
            outs=[_out_ap],
            transpose=False,
            num_idxs=num_idxs,
            elem_size=elem_size,
            stride_bytes_256=stride_bytes_256,
            gen_mode=0,
            single_packet=False,
            queue_num=queue_num,
            sbuf_tokens_per_rank=0,
            sbuf_free_dim_per_rank=0,
            sbuf_free_dim_pad_per_rank=0,
            sbuf_byte_offset=0,
        )
    )


def _preprocess(edge):
    src = np.asarray(edge[0], dtype=np.int64)
    dst = np.asarray(edge[1], dtype=np.int64)

    trow = (dst // SH) * REG + (dst % SH)
    q_all = trow >> 15
    loc_all = trow & (QROWS - 1)

    per_core = []
    for c in range(NC_CORES):
        m = (src >= c * SH) & (src < (c + 1) * SH)
        s = src[m] - c * SH
        q = q_all[m]
        loc = loc_all[m]
        t = s // P
        o = np.lexsort((s, q, t))
        per_core.append((t[o], q[o], s[o], loc[o]))

    # unified columns per (tile, quarter)
    CTQ = np.zeros((T, NQ), np.int64)
    for c in range(NC_CORES):
        t, q, s, loc = per_core[c]
        cnt = np.bincount(t * NQ + q, minlength=T * NQ).reshape(T, NQ)
        CTQ = np.maximum(CTQ, (cnt + P - 1) // P)

    coffs = np.zeros(T + 1, np.int64)
    coffs[1:] = np.cumsum(CTQ.sum(axis=1))
    CTOT = int(coffs[-1])
    CMAX = int(CTQ.sum(axis=1).max())

    # segs[t] = [(q, colstart_within_tile, cols)]
    segs = []
    for t in range(T):
        lst = []
        cs = 0
        for q in range(NQ):
            if CTQ[t, q] > 0:
                lst.append((q, cs, int(CTQ[t, q])))
                cs += int(CTQ[t, q])
        segs.append(lst)

    # per-core index / nodeid arrays
    eidx16 = np.full((NC_CORES, 128, 8 * CTOT), PADLOC, np.int16)
    nid = np.zeros((NC_CORES, 128, CTOT), np.float32)
    for c in range(NC_CORES):
        t, q, s, loc = per_core[c]
        key = t * NQ + q
        starts = np.searchsorted(key, np.arange(T * NQ + 1))
        srel = s % P
        for ti in range(T):
            for (qi, cs, cols) in segs[ti]:
                a, b = starts[ti * NQ + qi], starts[ti * NQ + qi + 1]
                n = b - a
                npad = cols * P
                iv = np.full(npad, PADLOC, np.int64)
                iv[:n] = loc[a:b]
                # wrapped-in-16, replicated x8 storage
                arr = iv.reshape(cols * 8, 16).T.astype(np.int16)  # [16, 8c]
                g0 = coffs[ti] + cs
                eidx16[c][:, 8 * g0: 8 * (g0 + cols)] = np.tile(arr, (8, 1))
                rel = np.zeros(npad, np.int64)
                rel[:n] = srel[a:b]
                blk = rel.reshape(cols, P).T  # [p, col]
                nid[c][:, g0:g0 + cols] = blk

    return dict(coffs=coffs.tolist(), CTOT=CTOT, CMAX=CMAX, segs=segs,
                eidx16=eidx16, nid=nid.astype(BF16))


def _build(meta, variant="full"):
    import concourse.bacc as bacc
    import concourse.mybir as mybir
    from concourse.tile import TileContext
    from concourse.masks import make_identity

    dt = mybir.dt
    AF = mybir.ActivationFunctionType
    ALU = mybir.AluOpType

    segs = meta["segs"]
    coffs = meta["coffs"]
    CTOT = meta["CTOT"]
    CMAX = meta["CMAX"]

    nc = bacc.Bacc()

    xT = nc.declare_dram_parameter("xT", [256, POS], dt.bfloat16, isOutput=False)
    w0e_d = nc.declare_dram_parameter("w0e", [256, 259], dt.bfloat16, isOutput=False)
    w1e_d = nc.declare_dram_parameter("w1e", [256, 131], dt.bfloat16, isOutput=False)
    lw = nc.declare_dram_parameter("lw", [128, 40], dt.bfloat16, isOutput=False)
    lb = nc.declare_dram_parameter("lb", [128, 40], dt.float32, isOutput=False)
    eidx_d = nc.declare_dram_parameter("eidx16", [128, 8 * CTOT], dt.int16, isOutput=False)
    nid_d = nc.declare_dram_parameter("nid", [128, CTOT], dt.bfloat16, isOutput=False)
    iomat_d = nc.declare_dram_parameter("iomat", [128, 128], dt.bfloat16, isOutput=False)
    logits = nc.declare_dram_parameter("logits", [POS, 40], dt.float32, isOutput=True)

    sh0 = nc.dram_tensor("sh0", [REG, STEP0], dt.bfloat16)
    t0 = nc.dram_tensor("t0", [NC_CORES * REG, STEP0], dt.bfloat16, addr_space="Shared")
    sh1 = nc.dram_tensor("sh1", [REG, STEP1], dt.bfloat16)
    t1 = nc.dram_tensor("t1", [NC_CORES * REG, STEP1], dt.bfloat16, addr_space="Shared")
    srow0 = nc.dram_tensor("srow0", [1, POS], dt.bfloat16)
    srow1 = nc.dram_tensor("srow1", [1, POS], dt.bfloat16)

    rg = [list(range(NC_CORES))]

    with TileContext(nc) as tc:
        with (
            tc.tile_pool(name="const", bufs=1) as constp,
            tc.tile_pool(name="gpool", bufs=2) as gpool,
            tc.tile_pool(name="ipool", bufs=2) as ipool,
            tc.tile_pool(name="spool", bufs=2) as spool,
            tc.tile_pool(name="hpool", bufs=3) as hpool,
            tc.tile_pool(name="xpool", bufs=3) as xpool,
            tc.tile_pool(name="psA", bufs=3, space="PSUM") as psA,
            tc.tile_pool(name="psB", bufs=2, space="PSUM") as psB,
        ):
            # ---- resident constants ----
            w0a = constp.tile([128, 259], dt.bfloat16, tag="w0a")
            w0b = constp.tile([128, 259], dt.bfloat16, tag="w0b")
            w1a = constp.tile([128, 131], dt.bfloat16, tag="w1a")
            w1b = constp.tile([128, 131], dt.bfloat16, tag="w1b")
            lwt = constp.tile([128, 40], dt.bfloat16, tag="lwt")
            lbt = constp.tile([128, 40], dt.float32, tag="lbt")
            ident = constp.tile([128, 128], dt.float32, tag="ident")
            iomat = constp.tile([128, 128], dt.bfloat16, tag="iomat")
            srep0 = constp.tile([128, POS], dt.bfloat16, tag="srep0")
            srep1 = constp.tile([128, POS], dt.bfloat16, tag="srep1")
            pad0 = constp.tile([1, STEP0], dt.bfloat16, tag="pad0")
            pad1 = constp.tile([1, STEP1], dt.bfloat16, tag="pad1")

            nc.sync.dma_start(out=w0a[:], in_=w0e_d[0:128, :])
            nc.sync.dma_start(out=w0b[:], in_=w0e_d[128:256, :])
            nc.sync.dma_start(out=w1a[:], in_=w1e_d[0:128, :])
            nc.sync.dma_start(out=w1b[:], in_=w1e_d[128:256, :])
            nc.sync.dma_start(out=lwt[:], in_=lw[:, :])
            nc.sync.dma_start(out=lbt[:], in_=lb[:, :])
            nc.sync.dma_start(out=iomat[:], in_=iomat_d[:, :])
            make_identity(nc, ident[:])
            nc.gpsimd.memset(pad0[:], 0.0)
            nc.gpsimd.memset(pad0[:, 256:257], 1.0)
            nc.gpsimd.memset(pad0[:, 257:258], NEG)
            nc.gpsimd.memset(pad1[:], 0.0)
            nc.gpsimd.memset(pad1[:, 128:129], 1.0)
            nc.gpsimd.memset(pad1[:, 129:130], NEG)
            nc.sync.dma_start(out=sh0[PADLOC:PADLOC + 1, :], in_=pad0[:])
            nc.sync.dma_start(out=sh1[PADLOC:PADLOC + 1, :], in_=pad1[:])

            regcache = {}

            def nreg(v):
                if v not in regcache:
                    regcache[v] = nc.gpsimd.to_reg(v)
                return regcache[v]

            # ---- dense layer 0 ----
            def dense0(t):
                cols = slice(t * P, (t + 1) * P)
                xa = xpool.tile([128, 128], dt.bfloat16, tag="xa")
                xb = xpool.tile([128, 128], dt.bfloat16, tag="xb")
                nc.sync.dma_start(out=xa[:], in_=xT[0:128, cols])
                nc.sync.dma_start(out=xb[:], in_=xT[128:256, cols])
                ps = psA.tile([128, 259], dt.float32, tag="ps")
                nc.tensor.matmul(ps[:], lhsT=xa[:], rhs=w0a[:], start=True, stop=False)
                nc.tensor.matmul(ps[:], lhsT=xb[:], rhs=w0b[:], start=False, stop=True)
                hb = hpool.tile([128, ELEM0], dt.bfloat16, tag="hb0")
                nc.vector.tensor_copy(out=hb[:], in_=ps[:, 0:ELEM0])
                nc.gpsimd.memset(hb[:, 256:257], 1.0)
                sr = xpool.tile([128, 1], dt.bfloat16, tag="sr")
                nc.vector.tensor_copy(out=sr[:], in_=ps[:, 258:259])
                nc.sync.dma_start(out=srow0[0:1, t * P:(t + 1) * P], in_=sr[:])
                rows = min(SH - t * P, P)
                nc.sync.dma_start(out=sh0[t * P:t * P + rows, 0:ELEM0], in_=hb[:rows, :])

            # ---- dense layer 1 from transposed ho (fused into edge L0) ----
            def dense1(t, ta0, ta1):
                ps1f = psA.tile([128, 259], dt.float32, tag="ps")
                ps1 = ps1f[:, 0:131]
                nc.tensor.matmul(ps1, lhsT=ta0[:], rhs=w1a[:], start=True, stop=False)
                nc.tensor.matmul(ps1, lhsT=ta1[:], rhs=w1b[:], start=False, stop=True)
                h1b = hpool.tile([128, ELEM1], dt.bfloat16, tag="h1b")
                nc.vector.tensor_copy(out=h1b[:], in_=ps1[:, 0:ELEM1])
                nc.gpsimd.memset(h1b[:, 128:129], 1.0)
                sr = xpool.tile([128, 1], dt.bfloat16, tag="sr")
                nc.vector.tensor_copy(out=sr[:], in_=ps1[:, 130:131])
                nc.sync.dma_start(out=srow1[0:1, t * P:(t + 1) * P], in_=sr[:])
                rows = min(SH - t * P, P)
                nc.sync.dma_start(out=sh1[t * P:t * P + rows, 0:ELEM1], in_=h1b[:rows, :])

            # ---- edge layer ----
            def edge_layer(table, ELEM, STEP, RH, srow_d, srep, emit, stage=9):
                dh = RH - 1
                nc.sync.dma_start(
                    out=srep[:], in_=srow_d[0:1, 0:POS].to_broadcast([128, POS]))
                for t in range(T):
                    co = coffs[t]
                    C = coffs[t + 1] - co
                    idx = ipool.tile([128, 8 * CMAX], dt.int16, tag="idx")
                    nc.sync.dma_start(out=idx[:, 0:8 * C],
                                      in_=eidx_d[:, 8 * co:8 * (co + C)])
                    nidt = ipool.tile([128, CMAX], dt.bfloat16, tag="nid")
                    nc.sync.dma_start(out=nidt[:, 0:C], in_=nid_d[:, co:co + C])
                    G = gpool.tile([128, CMAX * ELEM], dt.bfloat16, tag="g")
                    for (q, cs, colsq) in (segs[t] if stage >= 1 else []):
                        _dma_gather_raw(
                            nc.gpsimd,
                            out_ap=G[:, cs * ELEM:(cs + colsq) * ELEM].rearrange(
                                "p (c e) -> p c e", e=ELEM),
                            in_ap=table[q * QROWS:(q + 1) * QROWS, 0:ELEM],
                            idxs_ap=idx[:, 8 * cs:8 * (cs + colsq)],
                            num_idxs=colsq * P,
                            num_idxs_reg=nreg(colsq * P),
                            elem_size=ELEM,
                            elem_step=STEP,
                        )
                    if stage < 2:
                        zt2 = hpool.tile([128, 40], dt.float32, tag="lgo")
                        src_ap = G[:, 0:40] if stage >= 1 else srep[:, 0:40]
                        nc.vector.tensor_copy(out=zt2[:], in_=src_ap)
                        nc.sync.dma_start(out=logits[t * P:(t + 1) * P, :], in_=zt2[:])
                        continue
                    # selection matrices (absolute node ids, full width)
                    sel = spool.tile([128, CMAX * 128], dt.bfloat16, tag="sel")
                    nc.vector.tensor_tensor(
                        out=sel[:, 0:C * 128].rearrange("p (c w) -> p c w", w=128),
                        in0=nidt[:, 0:C].rearrange("p (c w) -> p c w", w=1)
                            .to_broadcast([128, C, 128]),
                        in1=iomat[:].rearrange("p (c w) -> p c w", c=1)
                            .to_broadcast([128, C, 128]),
                        op=ALU.is_equal,
                    )
                    if stage < 3:
                        zt2 = hpool.tile([128, 40], dt.float32, tag="lgo")
                        nc.vector.tensor_copy(out=zt2[:], in_=sel[:, 0:40])
                        nc.sync.dma_start(out=logits[t * P:(t + 1) * P, :], in_=zt2[:])
                        continue
                    # scores
                    g3 = G[:, 0:C * ELEM].rearrange("p (c e) -> p c e", e=ELEM)
                    sd = spool.tile([128, CMAX], dt.float32, tag="sd")
                    nc.scalar.copy(
                        out=sd[:, 0:C].rearrange("p (c o) -> p c o", o=1),
                        in_=g3[:, :, ELEM - 1:ELEM],
                    )
                    w2 = spool.tile([128, CMAX * 128], dt.bfloat16, tag="w2")
                    nc.vector.tensor_tensor(
                        out=w2[:, 0:C * 128].rearrange("p (c w) -> p c w", w=128),
                        in0=sel[:, 0:C * 128].rearrange("p (c w) -> p c w", w=128),
                        in1=srep[:, t * P:(t + 1) * P].rearrange(
                            "p (c w) -> p c w", c=1).to_broadcast([128, C, 128]),
                        op=ALU.mult,
                    )
                    srcsum = spool.tile([128, CMAX], dt.float32, tag="srcsum")
                    nc.vector.tensor_reduce(
                        out=srcsum[:, 0:C],
                        in_=w2[:, 0:C * 128].rearrange("p (c w) -> p c w", w=128),
                        axis=mybir.AxisListType.X, op=ALU.add,
                    )
                    sc0 = spool.tile([128, CMAX], dt.float32, tag="sc0")
                    nc.vector.tensor_tensor(
                        out=sc0[:, 0:C], in0=sd[:, 0:C], in1=srcsum[:, 0:C],
                        op=ALU.add)
                    sc = spool.tile([128, CMAX], dt.float32, tag="sc")
                    nc.vector.scalar_tensor_tensor(
                        out=sc[:, 0:C], in0=sc0[:, 0:C], scalar=ALPHA,
                        in1=sc0[:, 0:C], op0=ALU.mult, op1=ALU.max,
                    )
                    ebf = spool.tile([128, CMAX], dt.bfloat16, tag="ebf")
                    nc.scalar.activation(
                        out=ebf[:, 0:C], in_=sc[:, 0:C], func=AF.Exp,
                        bias=0.0, scale=1.0)
                    if stage < 4:
                        zt2 = hpool.tile([128, 40], dt.float32, tag="lgo")
                        nc.vector.tensor_copy(out=zt2[:], in_=ebf[:, 0:CMAX].to_broadcast([128, CMAX])[:, 0:40])
                        nc.sync.dma_start(out=logits[t * P:(t + 1) * P, :], in_=zt2[:])
                        continue
                    selw = spool.tile([128, CMAX * 128], dt.bfloat16, tag="selw")
                    nc.vector.tensor_tensor(
                        out=selw[:, 0:C * 128].rearrange("p (c w) -> p c w", w=128),
                        in0=sel[:, 0:C * 128].rearrange("p (c w) -> p c w", w=128),
                        in1=ebf[:, 0:C].rearrange("p (c w) -> p c w", w=1)
                            .to_broadcast([128, C, 128]),
                        op=ALU.mult,
                    )
                    # aggregation
                    acc = psB.tile([128, RH0], dt.float32, tag="acc")
                    for c in range(C):
                        nc.tensor.matmul(
                            acc[:, 0:RH],
                            lhsT=selw[:, c * 128:(c + 1) * 128],
                            rhs=G[:, c * ELEM:c * ELEM + RH],
                            start=(c == 0), stop=(c == C - 1),
                        )
                    z1 = spool.tile([128, 1], dt.float32, tag="z1")
                    nc.vector.tensor_scalar_add(z1[:], acc[:, RH - 1:RH], 1e-30)
                    rz = spool.tile([128, 1], dt.float32, tag="rz")
                    nc.vector.reciprocal(rz[:], z1[:])
                    hn = hpool.tile([128, dh], dt.float32, tag="hn")
                    nc.scalar.activation(
                        out=hn[:], in_=acc[:, 0:dh], func=AF.Copy,
                        bias=0.0, scale=rz[:, 0:1],
                    )
                    tneg = hpool.tile([128, dh], dt.float32, tag="tneg")
                    nc.vector.tensor_scalar_min(tneg[:], hn[:], 0.0)
                    expm = hpool.tile([128, dh], dt.float32, tag="expm")
                    nc.scalar.activation(out=expm[:], in_=tneg[:], func=AF.Exp, bias=0.0)
                    ho = hpool.tile([128, dh], dt.float32, tag="ho")
                    nc.vector.scalar_tensor_tensor(
                        out=ho[:], in0=expm[:], scalar=-1.0, in1=hn[:],
                        op0=ALU.add, op1=ALU.max,
                    )
                    emit(t, ho)

            # ---- layer-0 emit: transpose + dense1 ----
            def emit0(t, ho):
                tas = []
                for half in range(2):
                    ptf = psA.tile([128, 259], dt.float32, tag="ps")
                    pt = ptf[:, 0:128]
                    nc.tensor.transpose(
                        pt, ho[:, half * 128:(half + 1) * 128], ident[:])
                    ta = xpool.tile([128, 128], dt.bfloat16, tag="ta")
                    nc.scalar.copy(out=ta[:], in_=pt)
                    tas.append(ta)
                dense1(t, tas[0], tas[1])

            # ---- layer-1 emit: linear head + log_softmax ----
            def emit1(t, ho):
                ptf = psA.tile([128, 259], dt.float32, tag="ps")
                pt = ptf[:, 0:128]
                nc.tensor.transpose(pt, ho[:, 0:128], ident[:])
                h1T = xpool.tile([128, 128], dt.bfloat16, tag="ta")
                nc.scalar.copy(out=h1T[:], in_=pt)
                ps40f = psA.tile([128, 259], dt.float32, tag="ps")
                ps40 = ps40f[:, 0:40]
                nc.tensor.matmul(ps40, lhsT=h1T[:], rhs=lwt[:], start=True, stop=True)
                lg = hpool.tile([128, 40], dt.float32, tag="lg")
                nc.vector.tensor_tensor(
                    out=lg[:], in0=ps40, in1=lbt[:], op=ALU.add)
                m4 = spool.tile([128, 1], dt.float32, tag="m4")
                nc.vector.reduce_max(out=m4[:], in_=lg[:], axis=mybir.AxisListType.X)
                negm4 = spool.tile([128, 1], dt.float32, tag="negm4")
                nc.vector.tensor_scalar_mul(negm4[:], m4[:], -1.0)
                e4 = hpool.tile([128, 40], dt.float32, tag="e4")
                z4 = spool.tile([128, 1], dt.float32, tag="z4")
                nc.scalar.activation(
                    out=e4[:], in_=lg[:], func=AF.Exp,
                    bias=negm4[:, 0:1], scale=1.0, accum_out=z4[:, 0:1],
                )
                lnz = spool.tile([128, 1], dt.float32, tag="lnz")
                nc.scalar.activation(out=lnz[:], in_=z4[:], func=AF.Ln, bias=0.0)
                lgo = hpool.tile([128, 40], dt.float32, tag="lgo")
                nc.vector.tensor_scalar(
                    out=lgo[:], in0=lg[:], scalar1=negm4[:, 0:1],
                    scalar2=lnz[:, 0:1], op0=ALU.add, op1=ALU.subtract,
                )
                nc.sync.dma_start(out=logits[t * P:(t + 1) * P, :], in_=lgo[:])

            def final_dummy():
                zt = hpool.tile([128, 40], dt.float32, tag="lgo")
                nc.gpsimd.memset(zt[:], 0.0)
                for t in range(T):
                    nc.sync.dma_start(out=logits[t * P:(t + 1) * P, :], in_=zt[:])

            def emit_dbg(t, ho):
                lgo = hpool.tile([128, 40], dt.float32, tag="lgo")
                nc.vector.tensor_copy(out=lgo[:], in_=ho[:, 0:40])
                nc.sync.dma_start(out=logits[t * P:(t + 1) * P, :], in_=lgo[:])

            for t in range(T):
                dense0(t)
            nc.gpsimd.collective_compute(
                "AllGather", mybir.AluOpType.bypass,
                ins=[sh0[:]], outs=[t0[:]], replica_groups=rg,
            )
            if variant == "v0":
                final_dummy()
            elif variant.startswith("v1"):
                stage = int(variant[2:]) if len(variant) > 2 else 9
                edge_layer(t0, ELEM0, STEP0, RH0, srow0, srep0, emit_dbg, stage=stage)
            else:
                edge_layer(t0, ELEM0, STEP0, RH0, srow0, srep0, emit0)
                nc.gpsimd.collective_compute(
                    "AllGather", mybir.AluOpType.bypass,
                    ins=[sh1[:]], outs=[t1[:]], replica_groups=rg,
                )
                if variant == "v2":
                    final_dummy()
                else:
                    edge_layer(t1, ELEM1, STEP1, RH1, srow1, srep1, emit1)

    nc.finalize()
    return nc


def build_all(inputs):
    x = np.ascontiguousarray(np.asarray(inputs["x"], dtype=np.float32))
    edge = np.asarray(inputs["edge"])
    W0 = np.asarray(inputs["W0"], dtype=np.float32)
    a0 = np.asarray(inputs["a0"], dtype=np.float32)
    W1 = np.asarray(inputs["W1"], dtype=np.float32)
    a1 = np.asarray(inputs["a1"], dtype=np.float32)
    lin_w = np.asarray(inputs["lin_w"], dtype=np.float32)
    lin_b = np.asarray(inputs["lin_b"], dtype=np.float32)

    pre = _preprocess(edge)

    # w0e: [W0 | 0 (-> 1.0 col) | W0 a_dst | W0 a_src]
    w0e = np.concatenate(
        [W0, np.zeros((256, 1), np.float32), W0 @ a0[256:], W0 @ a0[:256]], axis=1)
    w1e = np.concatenate(
        [W1, np.zeros((256, 1), np.float32), W1 @ a1[128:], W1 @ a1[:128]], axis=1)
    lb_rep = np.tile(lin_b[None, :], (128, 1)).astype(np.float32)
    iomat = np.tile(np.arange(128, dtype=np.float32)[None, :], (128, 1)).astype(BF16)

    in_maps = []
    for c in range(NC_CORES):
        xTc = np.zeros((256, POS), np.float32)
        xTc[:, :SH] = x[c * SH:(c + 1) * SH].T
        in_maps.append({
            "xT": xTc.astype(BF16),
            "w0e": w0e.astype(BF16), "w1e": w1e.astype(BF16),
            "lw": lin_w.astype(BF16), "lb": lb_rep,
            "eidx16": pre["eidx16"][c],
            "nid": pre["nid"][c],
            "iomat": iomat,
        })

    import os
    nc = _build(pre, variant=os.environ.get("KVARIANT", "full"))
    return nc, in_maps, pre


def _assemble(results, pre):
    out = np.empty((N, 40), np.float32)
    for c in range(NC_CORES):
        out[c * SH:(c + 1) * SH] = results[c]["logits"][:SH]
    return out


def _ensure_device(max_tries=8, sleep_s=10.0):
    import time
    import jax

    for i in range(max_tries):
        try:
            a = jax.device_put(np.ones(8, np.float32))
            jax.block_until_ready(a + 1)
            return
        except Exception:  # noqa: BLE001
            if i == max_tries - 1:
                raise
            time.sleep(sleep_s)


def kernel(**inputs) -> np.ndarray:
    import time
    from concourse.bass_utils import run_bass_kernel_spmd

    nc, in_maps, pre = build_all(inputs)
    _ensure_device()
    last = None
    for _ in range(3):
        try:
            res = run_bass_kernel_spmd(nc, in_maps, list(range(NC_CORES)))
            return _assemble(res.results, pre)
        except Exception as e:  # noqa: BLE001
            last = e
            time.sleep(15.0)
            _ensure_device()
    raise last
